# revision 1
# baseline (speedup 1.0000x reference)
"""GRU-style segmented-scan aggregator for Trainium2 (8 NeuronCores).

Reference computes, per node n with messages m_1..m_L sorted by time t:
    h <- W @ (m + h) + b   starting from h = 0
and returns the final h per node (zeros for empty nodes).

Because every step uses the SAME matrix W, the final state has the closed
form (h_0 = 0):
    h = sum_{k=0}^{L-1} W^{k+1} m_{(L-1-k)}  +  S_L b,   S_L = sum_{p<L} W^p
i.e. the k-th message FROM THE END is hit by W^{k+1}.  This turns the
sequential scan into independent batched matmuls against precomputed powers
of W -- ideal for the PE array.

Device layout (per core, SPMD over 8 cores):
  - nodes are sorted by message count (desc) and dealt round-robin to cores;
    each core owns <=1024 node slots, columns of a [256 feat x 1024] H^T
    accumulator kept in PSUM (2 chunks of 128 partitions).
  - step k multiplies W^{k+1} (lhsT, 4 chunks of 128x128) with the k-th-from-
    end messages of the first n_k slots (rhs, features on partitions), and
    accumulates into PSUM.  n_k shrinks as shorter segments are exhausted, so
    there is ~0% padding.
  - the bias term S_L b per node is added when copying PSUM -> SBUF -> HBM.

Host does the (cheap) data marshalling: lexsort by (index, t), gather into
the k-major column layout, precompute W powers in fp64, scatter results back.
"""

import numpy as np

import concourse.bass as bass
import concourse.mybir as mybir
from concourse import tile
from concourse.bass_utils import run_bass_kernel_spmd
import bass_rust

_N_PROCS = 27


class _SplitDrainTC(tile.TileContext):
    """TileContext whose kernel-tail drain is split into one drain per proc.

    The walrus build in this container rejects instructions carrying more
    than one sync wait; the stock tail drain waits on every proc at once.
    Emitting a chain of drains, each waiting on a single semaphore, is
    semantically identical (all procs quiesced before the exit barrier).
    """

    def _drain_and_barrier(self, tick_clock, wait_clock):
        gc = tick_clock.global_clock
        for p in range(_N_PROCS):
            if gc[p] <= 0:
                continue
            d = self.nc.sync.drain()
            vc = bass_rust.VectorClock(
                [gc[q] if q == p else 0 for q in range(_N_PROCS)])
            wait_clock.add_sem_waits(d.ins, bass_rust.ScopedClock({None: vc}))
        assert self.sems is not None
        popped = self.nc._tile_sem_poison_stack.pop()
        assert popped is self._sem_poison
        self.nc.all_engine_barrier()
        self.nc.clear_and_free_semaphores(list(self.sems.allocated().values()))
        self.nc.all_engine_barrier()

N_CORES = 8
DIM = 256
SLOTS = 1024  # node slots per core == PSUM accumulator width

_NC_CACHE: dict = {}


def _build_nc(K0: int, n_k: tuple, Cdev: int):
    """Build the Bass program for one core (shared by all 8 via SPMD).

    This walrus build accepts at most ONE sync wait per instruction, so the
    kernel is written with zero SBUF-slot reuse (every stream block gets its
    own tile; a reused slot would need WAR+WAW = 2 waits on its DMA) and the
    bias is injected via identity matmul instead of a DVE add (which would
    carry PE + DMA = 2 waits).
    """
    f32 = mybir.dt.float32
    nc = bass.Bass()

    # merged stream: per step k<K0, block = [512 weight cols | n_k hi | n_k lo];
    # final block = [128 identity cols | btermT chunk0 (1024) | chunk1 (1024)]
    Q = K0 * 512 + 2 * Cdev + 128 + 2 * SLOTS
    mw = nc.dram_tensor("mw", [128, Q], f32, kind="ExternalInput")
    out = nc.dram_tensor("out", [128, 2 * SLOTS], f32, kind="ExternalOutput")

    with _SplitDrainTC(nc) as tc:
        with (
            tc.tile_pool(name="m", bufs=1) as mpool,
            tc.tile_pool(name="misc", bufs=1) as miscpool,
            tc.tile_pool(name="ps", bufs=1, space="PSUM") as pspool,
        ):
            phs = [pspool.tile([128, SLOTS], f32, tag=f"ph{i}", name=f"ph{i}")
                   for i in range(2)]
            q = 0
            for k in range(K0):
                nk = n_k[k]
                blk = 512 + 2 * nk
                mk = mpool.tile([128, blk], f32, tag=f"mk{k}", name=f"mk{k}")
                nc.sync.dma_start(mk[:], mw[:, q:q + blk])
                for i in range(2):          # output feature chunk (PSUM partitions)
                    for j in range(2):      # contraction chunk
                        wt = mk[:, j * 256 + i * 128: j * 256 + (i + 1) * 128]
                        for s in range(0, nk, 512):
                            e = min(nk, s + 512)
                            nc.tensor.matmul(
                                phs[i][:, s:e], wt,
                                mk[:, 512 + j * nk + s: 512 + j * nk + e],
                                start=(k == 0 and j == 0),
                                stop=False,
                                skip_group_check=True,
                            )
                q += blk

            # bias (+ host-folded tail) via identity matmul: psum_i += I.T @ btT_i
            fb = mpool.tile([128, 128 + 2 * SLOTS], f32, tag="fb", name="fb")
            nc.sync.dma_start(fb[:], mw[:, q:q + 128 + 2 * SLOTS])
            for i in range(2):
                ident = fb[:, 0:128]
                for s in range(0, SLOTS, 512):
                    nc.tensor.matmul(
                        phs[i][:, s:s + 512], ident,
                        fb[:, 128 + i * SLOTS + s: 128 + i * SLOTS + s + 512],
                        start=False, stop=(s + 512 >= SLOTS),
                        skip_group_check=True,
                    )

            # writeback: PSUM -> SBUF copy (waits PE only), DMA out via SWDGE
            # (fresh DMASW lanes -> single producer wait)
            for i in range(2):
                ot = miscpool.tile([128, SLOTS], f32, tag=f"ot{i}", name=f"ot{i}")
                nc.vector.tensor_copy(ot[:], phs[i][:])
                nc.gpsimd.dma_start(out[:, i * SLOTS:(i + 1) * SLOTS], ot[:])
    return nc


def _prepare(msg, index, t, dim_size, W, b):
    """Host-side marshalling. Returns (in_maps, node_ids, schedule key)."""
    E, D = msg.shape
    counts = np.bincount(index, minlength=dim_size)
    order = np.lexsort((t, index))            # stable: primary index, secondary t
    msg_sorted = msg[order]                   # [E, D] grouped by node, t-ascending
    seg_starts = np.zeros(dim_size, np.int64)
    seg_starts[1:] = np.cumsum(counts)[:-1]

    nodesort = np.argsort(-counts, kind="stable")
    nz = nodesort[counts[nodesort] > 0]
    per_core = -(-len(nz) // N_CORES)
    assert per_core <= SLOTS, f"too many nodes per core: {per_core}"

    node_ids = np.full((N_CORES, SLOTS), -1, np.int64)
    for c in range(N_CORES):
        ids = nz[c::N_CORES]
        node_ids[c, :len(ids)] = ids
    cc = np.where(node_ids >= 0, counts[np.maximum(node_ids, 0)], 0)  # [8, SLOTS]

    Lmax = int(cc.max())
    n_k = tuple(int((cc > k).sum(axis=1).max()) for k in range(Lmax))

    # device handles steps k < K0; the tail (k >= K0, ~1-2% of messages) is
    # folded into the per-node bias term on the host.  K0 chosen so the
    # no-slot-reuse SBUF footprint fits (~<=176 KiB/partition).
    K0 = Lmax
    while K0 > 1 and (K0 * 512 + 2 * sum(n_k[:K0]) + 128 + 2 * SLOTS) > 44000:
        K0 -= 1
    Cdev = int(sum(n_k[:K0]))

    # column -> position in msg_sorted (or -1 = zero pad), k-major layout
    rowidx = np.full((N_CORES, Cdev), -1, np.int64)
    off = 0
    for k in range(K0):
        nk = n_k[k]
        nid = node_ids[:, :nk]
        ck = cc[:, :nk]
        active = k < ck
        pos = seg_starts[np.maximum(nid, 0)] + ck - 1 - k
        rowidx[:, off:off + nk] = np.where(active, pos, -1)
        off += nk

    # weights: powers of W in fp64, stored transposed (lhsT chunks).
    # wfull per k: cols [0,256) = (W^{k+1}).T rows 0:128 (j=0 chunk),
    #              cols [256,512) = rows 128:256 (j=1 chunk).
    Wd = W.astype(np.float64)
    bd = b.astype(np.float64)
    wfull = np.empty((128, K0 * 512), np.float32)
    s_table = np.zeros((Lmax + 1, D), np.float64)   # s_p = S_p b
    Wpows = []                                      # W^{k+1} (fp64), k = 0..Lmax-1
    P = Wd.copy()
    for k in range(Lmax):
        if k < K0:
            WT = P.T.astype(np.float32)             # (W^{k+1}).T
            wfull[:, k * 512:k * 512 + 256] = WT[:128, :]
            wfull[:, k * 512 + 256:(k + 1) * 512] = WT[128:, :]
        Wpows.append(P)
        s_table[k + 1] = Wd @ s_table[k] + bd
        P = P @ Wd

    # per-(core, slot) bias term: S_L b plus host-folded tail contributions
    bterm = s_table[cc]                              # [8, SLOTS, 256] fp64
    for k in range(K0, Lmax):
        nk = n_k[k]
        act = k < cc[:, :nk]                         # [8, nk]
        cs, ss = np.nonzero(act)
        pos = seg_starts[node_ids[cs, ss]] + cc[cs, ss] - 1 - k
        Y = msg_sorted[pos].astype(np.float64) @ Wpows[k].T
        bterm[cs, ss] += Y
    bterm32 = bterm.astype(np.float32)

    ident = np.zeros((128, 128), np.float32)
    np.fill_diagonal(ident, 1.0)

    Q = K0 * 512 + 2 * Cdev + 128 + 2 * SLOTS
    in_maps = []
    for c in range(N_CORES):
        ri = rowidx[c]
        Mg = msg_sorted[np.maximum(ri, 0)]
        Mg[ri < 0] = 0.0                             # [Cdev, 256]
        hi = Mg[:, :128].T                           # [128, Cdev]
        lo = Mg[:, 128:].T
        mwb = np.empty((128, Q), np.float32)
        off = 0
        q = 0
        for k in range(K0):
            nk = n_k[k]
            mwb[:, q:q + 512] = wfull[:, k * 512:(k + 1) * 512]
            mwb[:, q + 512:q + 512 + nk] = hi[:, off:off + nk]
            mwb[:, q + 512 + nk:q + 512 + 2 * nk] = lo[:, off:off + nk]
            off += nk
            q += 512 + 2 * nk
        mwb[:, q:q + 128] = ident
        mwb[:, q + 128:q + 128 + SLOTS] = bterm32[c, :, :128].T
        mwb[:, q + 128 + SLOTS:q + 128 + 2 * SLOTS] = bterm32[c, :, 128:].T
        in_maps.append({"mw": mwb})
    return in_maps, node_ids, (K0, n_k[:K0], Cdev)


def _run(inputs: dict, trace: bool = False, **run_kwargs):
    msg = np.ascontiguousarray(np.asarray(inputs["msg"], dtype=np.float32))
    index = np.asarray(inputs["index"]).astype(np.int64)
    t = np.asarray(inputs["t"], dtype=np.float32)
    W = np.asarray(inputs["W"], dtype=np.float32)
    b = np.asarray(inputs["b"], dtype=np.float32)
    dim_size = int(inputs["dim_size"])

    in_maps, node_ids, key = _prepare(msg, index, t, dim_size, W, b)
    K0, n_k, Cdev = key
    if key not in _NC_CACHE:
        _NC_CACHE[key] = _build_nc(K0, n_k, Cdev)
    nc = _NC_CACHE[key]

    res = run_bass_kernel_spmd(nc, in_maps, list(range(N_CORES)),
                               trace=trace, **run_kwargs)

    hidden = np.zeros((dim_size, DIM), np.float32)
    for c in range(N_CORES):
        o = res.results[c]["out"]                    # [128, 2*SLOTS]
        hc = np.concatenate([o[:, :SLOTS], o[:, SLOTS:]], axis=0).T  # [SLOTS, 256]
        valid = node_ids[c] >= 0
        hidden[node_ids[c][valid]] = hc[valid]
    return hidden, res


def kernel(**inputs) -> np.ndarray:
    hidden, _ = _run(inputs, trace=False)
    return hidden



# revision 2
# speedup vs baseline: 2.5367x; 2.5367x over previous
"""GRU-style segmented-scan aggregator for Trainium2 (8 NeuronCores).

Reference computes, per node n with messages m_1..m_L sorted by time t:
    h <- W @ (m + h) + b   starting from h = 0
and returns the final h per node (zeros for empty nodes).

Because every step uses the SAME matrix W, the final state has the closed
form (h_0 = 0):
    h = sum_{k=0}^{L-1} W^{k+1} m_{(L-1-k)}  +  S_L b,   S_L = sum_{p<L} W^p
i.e. the k-th message FROM THE END is hit by W^{k+1}.  This turns the
sequential scan into independent batched matmuls against precomputed powers
of W -- ideal for the PE array.

Two-level blocking (B = 16) cuts the weight stream: split k = q*B + r, so
    W^{k+1} = W^{qB} @ W^{r+1}
Each device accumulates, per node, block partial sums
    c_q = sum_r W^{r+1} m_{(L-1-qB-r)}
using only the B matrices W^1..W^B (streamed once), then folds the q=1
block with one extra matmul  h += W^B @ c_1  (W^B is already on chip).
Messages at k >= 2B (~0.02% of all messages) are folded into the per-node
bias term on the host.

Everything streams in bf16 (messages, weights, bias), accumulates in fp32
PSUM: 2x less HBM traffic and 4x faster PE than the fp32 path (fp32 matmul
costs 4 cycles/column; bf16 costs 1).

Device layout (per core, SPMD over 8 cores):
  - nodes are sorted by message count (desc) and dealt round-robin to cores;
    each core owns <=1024 node slots.  PSUM holds, per 128-feature chunk i,
    a q=0 accumulator [128 x 1024] and a q=1 accumulator [128 x 512].
  - step r multiplies W^{r+1} (lhsT) against the r-th-from-block-end
    messages: n0_r columns into the q=0 region, n1_r columns into the q=1
    region.  n*_r shrink as shorter segments exhaust, so ~0% padding.
  - per step, two DMAs: [weights | hi-halves] then [lo-halves], so j=0
    matmuls start after half the block has landed.
  - the bias term S_L b per node is added via identity matmul when the
    PSUM accumulators are complete; writeback is PSUM -> SBUF (bf16) ->
    HBM, chunk by chunk so the last bias matmul overlaps the first store.

Host does the (cheap) data marshalling: lexsort by (index, t), gather into
the (r, q)-major column layout, precompute W powers in fp64, scatter
results back.
"""

import numpy as np
import ml_dtypes

import concourse.bass as bass
import concourse.mybir as mybir
from concourse import tile
from concourse.bass_utils import run_bass_kernel_spmd
import bass_rust

_N_PROCS = 27

BF16 = ml_dtypes.bfloat16


class _SplitDrainTC(tile.TileContext):
    """TileContext whose kernel-tail drain is split into one drain per proc.

    The walrus build in this container rejects instructions carrying more
    than one sync wait; the stock tail drain waits on every proc at once.
    Emitting a chain of drains, each waiting on a single semaphore, is
    semantically identical (all procs quiesced before the exit barrier).
    """

    def _drain_and_barrier(self, tick_clock, wait_clock):
        gc = tick_clock.global_clock
        for p in range(_N_PROCS):
            if gc[p] <= 0:
                continue
            d = self.nc.sync.drain()
            vc = bass_rust.VectorClock(
                [gc[q] if q == p else 0 for q in range(_N_PROCS)])
            wait_clock.add_sem_waits(d.ins, bass_rust.ScopedClock({None: vc}))
        assert self.sems is not None
        popped = self.nc._tile_sem_poison_stack.pop()
        assert popped is self._sem_poison
        self.nc.all_engine_barrier()
        self.nc.clear_and_free_semaphores(list(self.sems.allocated().values()))
        self.nc.all_engine_barrier()

N_CORES = 8
DIM = 256
SLOTS = 1024  # node slots per core == q0 PSUM accumulator width
BLK = 16      # power-blocking factor B

_NC_CACHE: dict = {}


def _build_nc(n0: tuple, n1: tuple, fb_off: int):
    """Build the Bass program for one core (shared by all 8 via SPMD).

    This walrus build accepts at most ONE sync wait per instruction, so the
    kernel is written with zero SBUF-slot reuse (every stream block gets its
    own tile; a reused slot would need WAR+WAW = 2 waits on its DMA) and the
    bias is injected via identity matmul instead of a DVE add (which would
    carry PE + DMA = 2 waits).
    """
    f32 = mybir.dt.float32
    bf16 = mybir.dt.bfloat16
    nc = bass.Bass()

    R0 = len(n0)
    R1 = len(n1)
    nq1 = n1[0] if R1 else 0

    Q = fb_off + 128 + 2 * SLOTS
    mw = nc.dram_tensor("mw", [128, Q], bf16, kind="ExternalInput")
    out = nc.dram_tensor("out", [128, 2 * SLOTS], bf16, kind="ExternalOutput")

    with _SplitDrainTC(nc) as tc:
        with (
            tc.tile_pool(name="m", bufs=1) as mpool,
            tc.tile_pool(name="misc", bufs=1) as miscpool,
            tc.tile_pool(name="ps", bufs=1, space="PSUM") as pspool,
        ):
            p0 = [pspool.tile([128, SLOTS], f32, tag=f"p0{i}", name=f"p0{i}")
                  for i in range(2)]
            p1 = [pspool.tile([128, 512], f32, tag=f"p1{i}", name=f"p1{i}")
                  for i in range(2)] if R1 else None

            wblk = None  # tile holding W^B (the r = BLK-1 stream block)
            q = 0
            for r in range(R0):
                a = n0[r]
                c = n1[r] if r < R1 else 0
                # hi part: [W^{r+1} 512 | q0 hi a | q1 hi c]
                hb = 512 + a + c
                mh = mpool.tile([128, hb], bf16, tag=f"mh{r}", name=f"mh{r}")
                nc.sync.dma_start(mh[:], mw[:, q:q + hb])
                # lo part: [q0 lo a | q1 lo c]
                lb = a + c
                ml = mpool.tile([128, lb], bf16, tag=f"ml{r}", name=f"ml{r}")
                nc.sync.dma_start(ml[:], mw[:, q + hb:q + hb + lb])
                if r == BLK - 1:
                    wblk = mh
                for j in range(2):          # contraction chunk
                    src = mh if j == 0 else ml
                    boff = 512 if j == 0 else 0
                    for i in range(2):      # output feature chunk
                        wt = mh[:, j * 256 + i * 128: j * 256 + (i + 1) * 128]
                        for s in range(0, a, 512):
                            e = min(a, s + 512)
                            nc.tensor.matmul(
                                p0[i][:, s:e], wt,
                                src[:, boff + s: boff + e],
                                start=(r == 0 and j == 0),
                                stop=False,
                                skip_group_check=True,
                            )
                        if c:
                            nc.tensor.matmul(
                                p1[i][:, 0:c], wt,
                                src[:, boff + a: boff + a + c],
                                start=(r == 0 and j == 0),
                                stop=False,
                                skip_group_check=True,
                            )
                q += hb + lb

            # fold block q=1:  p0 += (W^B)^T.T @ c1   (c1 = bf16 copy of p1)
            if R1:
                assert wblk is not None
                c1 = []
                for j in range(2):
                    t = miscpool.tile([128, nq1], bf16, tag=f"c1{j}",
                                      name=f"c1{j}")
                    nc.vector.tensor_copy(t[:], p1[j][:, 0:nq1])
                    c1.append(t)
                for i in range(2):
                    for j in range(2):
                        wt = wblk[:, j * 256 + i * 128: j * 256 + (i + 1) * 128]
                        nc.tensor.matmul(
                            p0[i][:, 0:nq1], wt, c1[j][:],
                            start=False, stop=False, skip_group_check=True,
                        )

            # bias (+ host-folded tail) via identity matmul: p0_i += I.T @ btT_i
            fb = mpool.tile([128, 128 + 2 * SLOTS], bf16, tag="fb", name="fb")
            nc.sync.dma_start(fb[:], mw[:, fb_off:fb_off + 128 + 2 * SLOTS])
            for i in range(2):
                ident = fb[:, 0:128]
                for s in range(0, SLOTS, 512):
                    nc.tensor.matmul(
                        p0[i][:, s:s + 512], ident,
                        fb[:, 128 + i * SLOTS + s: 128 + i * SLOTS + s + 512],
                        start=False, stop=(s + 512 >= SLOTS),
                        skip_group_check=True,
                    )
                # writeback: PSUM -> SBUF bf16 copy (waits PE only), DMA out
                # via SWDGE (fresh DMASW lanes -> single producer wait)
                ot = miscpool.tile([128, SLOTS], bf16, tag=f"ot{i}",
                                   name=f"ot{i}")
                nc.vector.tensor_copy(ot[:], p0[i][:])
                nc.gpsimd.dma_start(out[:, i * SLOTS:(i + 1) * SLOTS], ot[:])
    return nc


def _prepare(msg, index, t, dim_size, W, b):
    """Host-side marshalling. Returns (in_maps, node_ids, schedule key)."""
    E, D = msg.shape
    counts = np.bincount(index, minlength=dim_size)
    order = np.lexsort((t, index))            # stable: primary index, secondary t
    msg_sorted = msg[order]                   # [E, D] grouped by node, t-ascending
    seg_starts = np.zeros(dim_size, np.int64)
    seg_starts[1:] = np.cumsum(counts)[:-1]

    nodesort = np.argsort(-counts, kind="stable")
    nz = nodesort[counts[nodesort] > 0]
    per_core = -(-len(nz) // N_CORES)
    assert per_core <= SLOTS, f"too many nodes per core: {per_core}"

    node_ids = np.full((N_CORES, SLOTS), -1, np.int64)
    for c in range(N_CORES):
        ids = nz[c::N_CORES]
        node_ids[c, :len(ids)] = ids
    cc = np.where(node_ids >= 0, counts[np.maximum(node_ids, 0)], 0)  # [8, SLOTS]

    Lmax = int(cc.max())
    # device covers k < K0 = min(Lmax, 2*BLK); the tail (k >= K0, ~0.02% of
    # messages) is folded into the per-node bias term on the host.
    K0 = min(Lmax, 2 * BLK)
    R0 = min(BLK, K0)
    R1 = max(0, K0 - BLK)
    n0 = tuple(int((cc > r).sum(axis=1).max()) for r in range(R0))
    n1 = tuple(int((cc > BLK + r).sum(axis=1).max()) for r in range(R1))
    # q=1 region must fit one PSUM bank
    assert (not n1) or n1[0] <= 512, f"q1 region too wide: {n1[0]}"

    # column -> position in msg_sorted (or -1 = zero pad); stream block r is
    # [w 512 | q0 hi n0r | q1 hi n1r | q0 lo n0r | q1 lo n1r]
    def slot_pos(qb, r, nslots):
        nid = node_ids[:, :nslots]
        ck = cc[:, :nslots]
        active = (qb + r) < ck
        pos = seg_starts[np.maximum(nid, 0)] + ck - 1 - (qb + r)
        return np.where(active, pos, -1)

    # weights: powers of W in fp64, stored transposed (lhsT chunks).
    # w block per r: cols [0,256) = (W^{r+1}).T rows 0:128 (j=0 chunk),
    #                cols [256,512) = rows 128:256 (j=1 chunk).
    Wd = W.astype(np.float64)
    bd = b.astype(np.float64)
    wfull = np.empty((128, R0 * 512), np.float64)
    s_table = np.zeros((Lmax + 1, D), np.float64)   # s_p = S_p b
    Wpows = []                                      # W^{k+1} (fp64), k = 0..Lmax-1
    P = Wd.copy()
    for k in range(Lmax):
        if k < R0:
            WT = P.T
            wfull[:, k * 512:k * 512 + 256] = WT[:128, :]
            wfull[:, k * 512 + 256:(k + 1) * 512] = WT[128:, :]
        Wpows.append(P)
        s_table[k + 1] = Wd @ s_table[k] + bd
        P = P @ Wd

    # per-(core, slot) bias term: S_L b plus host-folded tail contributions
    bterm = s_table[cc]                              # [8, SLOTS, 256] fp64
    for k in range(K0, Lmax):
        act = k < cc
        cs, ss = np.nonzero(act)
        pos = seg_starts[node_ids[cs, ss]] + cc[cs, ss] - 1 - k
        Y = msg_sorted[pos].astype(np.float64) @ Wpows[k].T
        bterm[cs, ss] += Y
    bterm16 = bterm.astype(BF16)

    ident = np.zeros((128, 128), BF16)
    np.fill_diagonal(ident, 1.0)

    fb_off = sum(512 + 2 * (n0[r] + (n1[r] if r < R1 else 0))
                 for r in range(R0))
    Q = fb_off + 128 + 2 * SLOTS
    msg16 = msg_sorted.astype(BF16)
    wfull16 = wfull.astype(BF16)
    in_maps = []
    for c in range(N_CORES):
        mwb = np.zeros((128, Q), BF16)
        q = 0
        for r in range(R0):
            a = n0[r]
            cn = n1[r] if r < R1 else 0
            mwb[:, q:q + 512] = wfull16[:, r * 512:(r + 1) * 512]
            for qi, nslots in ((0, a),) + (((1, cn),) if cn else ()):
                ri = slot_pos(qi * BLK, r, nslots)[c]
                Mg = msg16[np.maximum(ri, 0)]
                Mg[ri < 0] = 0.0                     # [nslots, 256]
                off = q + 512 + (a if qi else 0)
                mwb[:, off:off + nslots] = Mg[:, :128].T
                mwb[:, off + a + cn:off + a + cn + nslots] = Mg[:, 128:].T
            q += 512 + 2 * (a + cn)
        assert q == fb_off
        mwb[:, q:q + 128] = ident
        mwb[:, q + 128:q + 128 + SLOTS] = bterm16[c, :, :128].T
        mwb[:, q + 128 + SLOTS:q + 128 + 2 * SLOTS] = bterm16[c, :, 128:].T
        in_maps.append({"mw": mwb})
    return in_maps, node_ids, (n0, n1, fb_off)


def _run(inputs: dict, trace: bool = False, **run_kwargs):
    msg = np.ascontiguousarray(np.asarray(inputs["msg"], dtype=np.float32))
    index = np.asarray(inputs["index"]).astype(np.int64)
    t = np.asarray(inputs["t"], dtype=np.float32)
    W = np.asarray(inputs["W"], dtype=np.float32)
    b = np.asarray(inputs["b"], dtype=np.float32)
    dim_size = int(inputs["dim_size"])

    in_maps, node_ids, key = _prepare(msg, index, t, dim_size, W, b)
    n0, n1, fb_off = key
    if key not in _NC_CACHE:
        _NC_CACHE[key] = _build_nc(n0, n1, fb_off)
    nc = _NC_CACHE[key]

    res = run_bass_kernel_spmd(nc, in_maps, list(range(N_CORES)),
                               trace=trace, **run_kwargs)

    hidden = np.zeros((dim_size, DIM), np.float32)
    for c in range(N_CORES):
        o = np.asarray(res.results[c]["out"], dtype=np.float32)  # [128, 2*SLOTS]
        hc = np.concatenate([o[:, :SLOTS], o[:, SLOTS:]], axis=0).T  # [SLOTS, 256]
        valid = node_ids[c] >= 0
        hidden[node_ids[c][valid]] = hc[valid]
    return hidden, res


def kernel(**inputs) -> np.ndarray:
    hidden, _ = _run(inputs, trace=False)
    return hidden


# revision 8
# speedup vs baseline: 2.5957x; 1.0232x over previous
"""GRU-style segmented-scan aggregator for Trainium2 (8 NeuronCores).

Reference computes, per node n with messages m_1..m_L sorted by time t:
    h <- W @ (m + h) + b   starting from h = 0
and returns the final h per node (zeros for empty nodes).

Because every step uses the SAME matrix W, the final state has the closed
form (h_0 = 0):
    h = sum_{k=0}^{L-1} W^{k+1} m_{(L-1-k)}  +  S_L b,   S_L = sum_{p<L} W^p
i.e. the k-th message FROM THE END is hit by W^{k+1}.  This turns the
sequential scan into independent batched matmuls against precomputed powers
of W -- ideal for the PE array.

Two tricks shrink the device work further:
  * Two-level blocking (B = 16): split k = q*B + r, so W^{k+1} = W^{qB} W^{r+1}.
    Each device accumulates block partials c_q with only W^1..W^B streamed,
    then folds the q=1 block with one extra matmul h += W^B c_1 (W^B is
    already on chip).  Messages at k >= 2B (~0.02%) fold into the host term.
  * Bias fold: h = sum W^{k+1} m_k + S_L b = sum W^{k+1} m'_k with
    m'_last = m_last + W^{-1} S_L b, so no separate bias pass exists on
    device at all (numerically safe here: ||W^{-1} S_L b|| <= ~4).

Everything streams in bf16 (1 PE cycle/column vs 4 for fp32; half the HBM
bytes), accumulates in fp32 PSUM.

Per-core schedule (SPMD over 8 cores; nodes sorted by message count desc and
dealt round-robin, <= 1024 slots/core):
  * PSUM: per 128-feature chunk i, accumulators p0a (slots 0-511), p0b
    (slots 512-1023), and p1 (q=1 block, <=512 slots).  One bank each.
  * step r: lhsT = W^{r+1}; rhs columns = r-th-from-block-end messages.
    n0_r / n1_r shrink as segments exhaust -- ~0% padding.
  * stream blocks are CONTIGUOUS dram tensors, two DMA paths that run ahead
    of the PE freely (no SBUF reuse): hi_r = [w | q1 hi | q1 lo | q0 hi] on
    the sync HWDGE queue, lo_r = [q0 lo] on the gpsimd SWDGE queue.  One
    sem wait per matmul (this walrus build allows at most one).
  * tail: c_1 -> bf16 SBUF (DVE), outer matmul W^B c_1 into p0a; meanwhile
    DVE drains p0b to SBUF and ACT stores it (slots 512+ take no part in
    the outer fold since only the 450 longest segments have a q=1 block).
    Final p0a copies run on DVE and ACT in parallel, stores on SWDGE.

Host does the (cheap) marshalling: lexsort by (index, t), gather into the
(r, q)-major column layout, precompute W powers in fp64, scatter back.
"""

import numpy as np
import ml_dtypes

import concourse.bass as bass
import concourse.mybir as mybir
from concourse import tile
from concourse.bass_utils import run_bass_kernel_spmd
import bass_rust

_N_PROCS = 27

BF16 = ml_dtypes.bfloat16


class _SplitDrainTC(tile.TileContext):
    """TileContext whose kernel-tail drain is split into one drain per proc.

    The walrus build in this container rejects instructions carrying more
    than one sync wait; the stock tail drain waits on every proc at once.
    Emitting a chain of drains, each waiting on a single semaphore, is
    semantically identical (all procs quiesced before the exit barrier).
    """

    def _drain_and_barrier(self, tick_clock, wait_clock):
        gc = tick_clock.global_clock
        for p in range(_N_PROCS):
            if gc[p] <= 0:
                continue
            d = self.nc.sync.drain()
            vc = bass_rust.VectorClock(
                [gc[q] if q == p else 0 for q in range(_N_PROCS)])
            wait_clock.add_sem_waits(d.ins, bass_rust.ScopedClock({None: vc}))
        assert self.sems is not None
        popped = self.nc._tile_sem_poison_stack.pop()
        assert popped is self._sem_poison
        self.nc.all_engine_barrier()
        self.nc.clear_and_free_semaphores(list(self.sems.allocated().values()))
        self.nc.all_engine_barrier()

N_CORES = 8
DIM = 256
SLOTS = 1024  # node slots per core
BLK = 16      # power-blocking factor B

_NC_CACHE: dict = {}


def _build_nc(n0: tuple, n1: tuple):
    """Build the Bass program for one core (shared by all 8 via SPMD)."""
    f32 = mybir.dt.float32
    bf16 = mybir.dt.bfloat16
    nc = bass.Bass()

    R0 = len(n0)
    R1 = len(n1)
    nq1 = n1[0] if R1 else 0
    # last r that touches the p0b bank (slots 512+)
    rb_last = max((r for r in range(R0) if n0[r] > 512), default=-1)

    # r = 0 is split so the first matmuls (weights + q1) start before the
    # bulk q0 columns land
    wq0 = nc.dram_tensor("wq0", [128, 512 + 2 * (n1[0] if R1 else 0)],
                         bf16, kind="ExternalInput")
    his = [nc.dram_tensor(f"hi{r}",
                          [128, (512 + 2 * (n1[r] if r < R1 else 0)) * (r > 0)
                           + n0[r]],
                          bf16, kind="ExternalInput") for r in range(R0)]
    los = [nc.dram_tensor(f"lo{r}", [128, n0[r]], bf16, kind="ExternalInput")
           for r in range(R0)]
    outs = {(i, half): nc.dram_tensor(f"o{'ab'[half]}{i}", [128, 512], bf16,
                                      kind="ExternalOutput")
            for i in range(2) for half in range(2)}

    with _SplitDrainTC(nc) as tc:
        with (
            tc.tile_pool(name="m", bufs=1) as mpool,
            tc.tile_pool(name="misc", bufs=1) as miscpool,
            tc.tile_pool(name="ps", bufs=1, space="PSUM") as pspool,
        ):
            p0a = [pspool.tile([128, 512], f32, tag=f"p0a{i}", name=f"p0a{i}")
                   for i in range(2)]
            p0b = [pspool.tile([128, 512], f32, tag=f"p0b{i}", name=f"p0b{i}")
                   for i in range(2)]
            p1 = [pspool.tile([128, 512], f32, tag=f"p1{j}", name=f"p1{j}")
                  for j in range(2)] if R1 else None

            wblk = None  # tile holding [W^B | q1] (the r = BLK-1 hi block)
            for r in range(R0):
                a = n0[r]
                c = n1[r] if r < R1 else 0
                if r == 0:
                    tw = mpool.tile([128, 512 + 2 * c], bf16,
                                    tag="tw0", name="tw0")
                    nc.sync.dma_start(tw[:], wq0[:])
                    th = mpool.tile([128, a], bf16, tag="th0", name="th0")
                    nc.sync.dma_start(th[:], his[0][:])
                    hoff = 0
                else:
                    tw = th = mpool.tile([128, 512 + 2 * c + a], bf16,
                                         tag=f"th{r}", name=f"th{r}")
                    nc.sync.dma_start(th[:], his[r][:])
                    hoff = 512 + 2 * c
                tl = mpool.tile([128, a], bf16, tag=f"tl{r}", name=f"tl{r}")
                nc.sync.dma_start(tl[:], los[r][:])
                if r == BLK - 1:
                    wblk = tw
                for i in range(2):          # output feature chunk
                    for j in range(2):      # contraction chunk
                        wt = tw[:, j * 256 + i * 128: j * 256 + (i + 1) * 128]
                        if c:
                            nc.tensor.matmul(
                                p1[i][:, 0:c], wt,
                                tw[:, 512 + j * c: 512 + (j + 1) * c],
                                start=(r == 0 and j == 0), stop=False,
                                skip_group_check=True,
                            )
                        src, boff = (th, hoff) if j == 0 else (tl, 0)
                        nc.tensor.matmul(
                            p0a[i][:, 0:min(a, 512)], wt,
                            src[:, boff: boff + min(a, 512)],
                            start=(r == 0 and j == 0), stop=False,
                            skip_group_check=True,
                        )
                        if a > 512:
                            nc.tensor.matmul(
                                p0b[i][:, 0:a - 512], wt,
                                src[:, boff + 512: boff + a],
                                start=(r == 0 and j == 0),
                                stop=(r == rb_last and i == 1 and j == 1),
                                skip_group_check=True,
                            )

            # c_1 -> bf16 SBUF (DVE); p0b drain (DVE) + store (SWDGE: fresh
            # DMASW lanes -> single producer wait) overlaps the outer matmuls
            c1 = []
            if R1:
                for j in range(2):
                    t = miscpool.tile([128, nq1], bf16, tag=f"c1{j}",
                                      name=f"c1{j}")
                    nc.vector.tensor_copy(t[:], p1[j][:, 0:nq1])
                    c1.append(t)
            otb = []
            for i in range(2):
                t = miscpool.tile([128, 512], bf16, tag=f"otb{i}",
                                  name=f"otb{i}")
                nc.vector.tensor_copy(t[:], p0b[i][:])
                otb.append(t)
                nc.gpsimd.dma_start(outs[(i, 1)][:], t[:])
            if R1:
                assert wblk is not None
                for i in range(2):
                    for j in range(2):
                        wt = wblk[:, j * 256 + i * 128: j * 256 + (i + 1) * 128]
                        nc.tensor.matmul(
                            p0a[i][:, 0:nq1], wt, c1[j][:],
                            start=False, stop=(i == 1 and j == 1),
                            skip_group_check=True,
                        )

            # final copies in parallel on DVE + ACT, stores on SWDGE
            ota0 = miscpool.tile([128, 512], bf16, tag="ota0", name="ota0")
            nc.vector.tensor_copy(ota0[:], p0a[0][:])
            ota1 = miscpool.tile([128, 512], bf16, tag="ota1", name="ota1")
            nc.scalar.copy(ota1[:], p0a[1][:])
            nc.gpsimd.dma_start(outs[(0, 0)][:], ota0[:])
            nc.gpsimd.dma_start(outs[(1, 0)][:], ota1[:])
    return nc


def _prepare(msg, index, t, dim_size, W, b):
    """Host-side marshalling. Returns (in_maps, node_ids, schedule key)."""
    E, D = msg.shape
    counts = np.bincount(index, minlength=dim_size)
    order = np.lexsort((t, index))            # stable: primary index, secondary t
    msg_sorted = msg[order]                   # [E, D] grouped by node, t-ascending
    seg_starts = np.zeros(dim_size, np.int64)
    seg_starts[1:] = np.cumsum(counts)[:-1]

    nodesort = np.argsort(-counts, kind="stable")
    nz = nodesort[counts[nodesort] > 0]
    per_core = -(-len(nz) // N_CORES)
    assert per_core <= SLOTS, f"too many nodes per core: {per_core}"

    node_ids = np.full((N_CORES, SLOTS), -1, np.int64)
    for c in range(N_CORES):
        ids = nz[c::N_CORES]
        node_ids[c, :len(ids)] = ids
    cc = np.where(node_ids >= 0, counts[np.maximum(node_ids, 0)], 0)  # [8, SLOTS]

    Lmax = int(cc.max())
    # device covers k < K0 = min(Lmax, 2*BLK); the tail (k >= K0, ~0.02% of
    # messages) is folded into the last-message term on the host.
    K0 = min(Lmax, 2 * BLK)
    R0 = min(BLK, K0)
    R1 = max(0, K0 - BLK)
    n0 = tuple(int((cc > r).sum(axis=1).max()) for r in range(R0))
    n1 = tuple(int((cc > BLK + r).sum(axis=1).max()) for r in range(R1))
    assert (not n1) or n1[0] <= 512, f"q1 region too wide: {n1[0]}"

    def slot_pos(qb, r, nslots):
        """msg_sorted row (or -1) per (core, slot<nslots) for step qb+r."""
        nid = node_ids[:, :nslots]
        ck = cc[:, :nslots]
        active = (qb + r) < ck
        pos = seg_starts[np.maximum(nid, 0)] + ck - 1 - (qb + r)
        return np.where(active, pos, -1)

    # weights: powers of W in fp64, stored transposed (lhsT chunks).
    # w block per r: cols [0,256) = (W^{r+1}).T rows 0:128 (j=0 chunk),
    #                cols [256,512) = rows 128:256 (j=1 chunk).
    Wd = W.astype(np.float64)
    bd = b.astype(np.float64)
    wfull = np.empty((128, R0 * 512), np.float64)
    s_table = np.zeros((Lmax + 1, D), np.float64)   # s_p = S_p b
    Wpows = []                                      # W^{k+1} (fp64), k = 0..Lmax-1
    P = Wd.copy()
    for k in range(Lmax):
        if k < R0:
            WT = P.T
            wfull[:, k * 512:k * 512 + 256] = WT[:128, :]
            wfull[:, k * 512 + 256:(k + 1) * 512] = WT[128:, :]
        Wpows.append(P)
        s_table[k + 1] = Wd @ s_table[k] + bd
        P = P @ Wd
    wfull16 = wfull.astype(BF16)

    # bias fold: the k=0 message of each slot gets + W^{-1} S_L b, plus the
    # host-folded tail  sum_{k>=K0} W^k m_k  (so W^1 applied on device
    # reproduces  S_L b + sum_{k>=K0} W^{k+1} m_k).
    WiS = np.linalg.solve(Wd, s_table.T).T          # [Lmax+1, 256]
    fold = WiS[cc]                                   # [8, SLOTS, 256] fp64
    for k in range(K0, Lmax):
        act = k < cc
        cs, ss = np.nonzero(act)
        pos = seg_starts[node_ids[cs, ss]] + cc[cs, ss] - 1 - k
        Wk = Wpows[k - 1] if k >= 1 else np.eye(D)   # W^k
        fold[cs, ss] += msg_sorted[pos].astype(np.float64) @ Wk.T

    msg16 = msg_sorted.astype(BF16)
    in_maps = [dict() for _ in range(N_CORES)]
    for r in range(R0):
        a = n0[r]
        cn = n1[r] if r < R1 else 0
        ri0 = slot_pos(0, r, a)                      # [8, a]
        woff = 0 if r == 0 else 512 + 2 * cn
        hi = np.zeros((N_CORES, 128, woff + a), BF16)
        lo = np.empty((N_CORES, 128, a), BF16)
        wq = np.zeros((N_CORES, 128, 512 + 2 * cn), BF16)
        wq[:, :, 0:512] = wfull16[None, :, r * 512:(r + 1) * 512]
        for c in range(N_CORES):
            ri = ri0[c]
            Mg = msg16[np.maximum(ri, 0)]
            Mg[ri < 0] = 0.0                         # [a, 256]
            if r == 0:
                Mg = (msg_sorted[np.maximum(ri, 0)].astype(np.float64)
                      + fold[c, :a]).astype(BF16)
                Mg[ri < 0] = 0.0
            hi[c, :, woff:] = Mg[:, :128].T
            lo[c] = Mg[:, 128:].T
            if cn:
                ri1 = slot_pos(BLK, r, cn)[c]
                Mg1 = msg16[np.maximum(ri1, 0)]
                Mg1[ri1 < 0] = 0.0
                wq[c, :, 512:512 + cn] = Mg1[:, :128].T
                wq[c, :, 512 + cn:512 + 2 * cn] = Mg1[:, 128:].T
            if r > 0:
                hi[c, :, 0:woff] = wq[c]
        for c in range(N_CORES):
            if r == 0:
                in_maps[c]["wq0"] = wq[c]
            in_maps[c][f"hi{r}"] = hi[c]
            in_maps[c][f"lo{r}"] = lo[c]
    return in_maps, node_ids, (n0, n1)


def _run(inputs: dict, trace: bool = False, **run_kwargs):
    msg = np.ascontiguousarray(np.asarray(inputs["msg"], dtype=np.float32))
    index = np.asarray(inputs["index"]).astype(np.int64)
    t = np.asarray(inputs["t"], dtype=np.float32)
    W = np.asarray(inputs["W"], dtype=np.float32)
    b = np.asarray(inputs["b"], dtype=np.float32)
    dim_size = int(inputs["dim_size"])

    in_maps, node_ids, key = _prepare(msg, index, t, dim_size, W, b)
    n0, n1 = key
    if key not in _NC_CACHE:
        _NC_CACHE[key] = _build_nc(n0, n1)
    nc = _NC_CACHE[key]

    res = run_bass_kernel_spmd(nc, in_maps, list(range(N_CORES)),
                               trace=trace, **run_kwargs)

    hidden = np.zeros((dim_size, DIM), np.float32)
    for c in range(N_CORES):
        r = res.results[c]
        o = np.concatenate(
            [np.concatenate([np.asarray(r["oa0"], np.float32),
                             np.asarray(r["ob0"], np.float32)], axis=1),
             np.concatenate([np.asarray(r["oa1"], np.float32),
                             np.asarray(r["ob1"], np.float32)], axis=1)],
            axis=0)                                   # [256, SLOTS]
        hc = o.T                                      # [SLOTS, 256]
        valid = node_ids[c] >= 0
        hidden[node_ids[c][valid]] = hc[valid]
    return hidden, res


def kernel(**inputs) -> np.ndarray:
    hidden, _ = _run(inputs, trace=False)
    return hidden
